# revision 6
# baseline (speedup 1.0000x reference)
# Trainium2 Bass kernel for nn_DistLoss: dist = 25-neighbor channel-L1
# distances -> tiny MLP (25->10->25, exact gelu) -> relu = dist_l1;
# loss assembled from dist_l1 + label-agreement mask.
#
# Sharding: data-parallel over (batch, image half): core k handles batch k//2,
# rows 128*(k%2) .. +128.  Inside a core, SBUF partitions = (column-group g,
# channel c): p = g*16 + c, so the channel reduction is a PE matmul over the
# partition axis and all neighbor shifts are pure free-dim AP offsets into a
# reflection-padded x tile prepared host-side.  |x - shift(x)| is computed for
# only 12 of the 24 non-center offsets; the mirror offset reuses the same
# absdiff grid at a shifted window (|a-b| symmetry).  The MLP runs on PE with
# block-diagonal packed weights (8 column-groups at once); gelu/relu/bias on
# the scalar engine.  dist_l1 is written HBM-contiguous in a device-friendly
# [g, o, r, c] layout and transposed to [B,H,W,25] on host.  The scalar loss
# factorizes as (n_pos*S_mismatch + n_neg*S_match)/numel^2 with plain sums, so
# it is assembled on host from dist_l1 and y.
import threading

import numpy as np
import ml_dtypes

B, C, H, W = 4, 16, 256, 256
ALPHA, BETA = 0.5, 2.0
N_CORES = 8
ROWS = 128           # output rows per core
G, GW = 8, 32        # column groups x width
XLO_R, XHI_R = -5, 132
XLO_C, XHI_C = -6, 37
XR = XHI_R - XLO_R + 1   # 138
XC = XHI_C - XLO_C + 1   # 44
NB = 8               # n-superblocks (each 16 rows x 32 cols = 512 pixels)
NBW = 512
BF16 = ml_dtypes.bfloat16


def _sorted_offsets():
    K = 11
    c0 = K // 2
    offs = set()
    for r in (1, 2, 5):
        for dy in (-r, 0, r):
            for dx in (-r, 0, r):
                offs.add((c0 + dy) * K + (c0 + dx))
    return [(v // K - c0, v % K - c0) for v in sorted(offs)]


OFFS = _sorted_offsets()          # center at index 12; OFFS[24-i] == -OFFS[i]
REPS = OFFS[13:]                  # 12 representatives (positive half)


def _grid_geom(dy, dx):
    """absdiff grid for rep (dy,dx): covers +window [0..127]x[0..31] and
    -window (grid at p-delta); col start/width padded even for DVE bf16
    2x-mode alignment."""
    lo_r = -dy
    n_r = 128 + dy
    if dx > 0:
        lo_c, hi_c = -dx, 31
    else:
        lo_c, hi_c = 0, 31 - dx
    if lo_c % 2 != 0:
        lo_c -= 1
    if (hi_c - lo_c + 1) % 2 != 0:
        hi_c += 1
    return lo_r, n_r, lo_c, hi_c - lo_c + 1


_lock = threading.Lock()
_cached = {}
_absdiff_op = None


def _get_absdiff_op():
    """Register a fused |a-b| custom DVE op: one Vector-engine pass instead of
    subtract + abs (abs_max is not a valid TRN2 TensorTensor ALU op)."""
    global _absdiff_op
    if _absdiff_op is not None:
        return _absdiff_op
    from concourse import dve_ops as dvo
    from concourse.dve_spec import Spec, Src0, Src1, maxx, lower, _has_src1
    from concourse.dve_uop import DveOpSpec

    name = "ABS_DIFF_ANT"
    spec = Spec(
        body=maxx(Src0 - Src1, Src1 - Src0),
        reference=lambda in0, in1, s0, s1, imm2: np.abs(
            in0.astype(np.float32) - in1.astype(np.float32)
        ),
    )
    op = dvo.DveOp(name, spec, subdim=False, uops_sha={})
    if name not in dvo._SUB_OPCODE_FOR_NAME:
        row = max(dvo._SUB_OPCODE_FOR_NAME.values()) + 1
        assert row < 0x20
        dvo._SUB_OPCODE_FOR_NAME[name] = row
        dvo.OPS.append(op)
        dvo.CUSTOM_DVE_SPECS[name] = spec
    row = dvo._SUB_OPCODE_FOR_NAME[name]
    for ver in ("v3",):
        s = DveOpSpec(
            name=name, opcode=row, uops=lower(spec, ver=ver), rd1_en=_has_src1(spec)
        ).sha(ver)
        op.uops_sha[ver] = s
    _absdiff_op = op
    return op


def _build_module(iters=1):
    import concourse.bacc as bacc
    import concourse.mybir as mybir
    from concourse.tile import TileContext
    from contextlib import ExitStack

    f32 = mybir.dt.float32
    bf16 = mybir.dt.bfloat16
    AF = mybir.ActivationFunctionType
    ALU = mybir.AluOpType

    absdiff = _get_absdiff_op()
    nc = bacc.Bacc(
        "TRN2",
        target_bir_lowering=False,
        debug=False,
        enable_asserts=False,
        num_devices=N_CORES,
    )

    xe_d = nc.dram_tensor("xe", [128, XR, XC], bf16, kind="ExternalInput").ap()
    xo_d = nc.dram_tensor("xo", [128, XR, XC], bf16, kind="ExternalInput").ap()
    ls_d = nc.dram_tensor("ls", [128, 24 * 96], bf16, kind="ExternalInput").ap()
    l1_d = nc.dram_tensor("l1", [96, 160], f32, kind="ExternalInput").ap()
    l2_d = nc.dram_tensor("l2", [80, 200], f32, kind="ExternalInput").ap()
    bias_d = nc.dram_tensor("bias", [104, 3], f32, kind="ExternalInput").ap()
    out_d = nc.dram_tensor("out", [G, 25, ROWS, GW], f32, kind="ExternalOutput").ap()

    with TileContext(nc) as tc, ExitStack() as ctx:
        const = ctx.enter_context(tc.tile_pool(name="const", bufs=1))
        xpool = ctx.enter_context(tc.tile_pool(name="x", bufs=1))
        adpool = ctx.enter_context(tc.tile_pool(name="ad", bufs=1))
        dwork = ctx.enter_context(tc.tile_pool(name="dwork", bufs=2))
        opool = ctx.enter_context(tc.tile_pool(name="o", bufs=3))
        psum1 = ctx.enter_context(tc.tile_pool(name="ps1", bufs=1, space="PSUM"))
        psum2 = ctx.enter_context(tc.tile_pool(name="ps2", bufs=2, space="PSUM"))

        ls_t = const.tile([128, 24 * 96], bf16)
        nc.sync.dma_start(ls_t[:], ls_d[:])
        l1_t = const.tile([96, 160], f32)
        nc.sync.dma_start(l1_t[:], l1_d[:])
        l2_t = const.tile([80, 200], f32)
        nc.sync.dma_start(l2_t[:], l2_d[:])
        bias_t = const.tile([104, 3], f32)
        nc.sync.dma_start(bias_t[:], bias_d[:])

        for _ in range(iters):
            xe_t = xpool.tile([128, XR, XC], bf16, tag="xe")
            nc.sync.dma_start(xe_t[:], xe_d[:])
            xo_t = xpool.tile([128, XR, XC], bf16, tag="xo")
            nc.sync.dma_start(xo_t[:], xo_d[:])

            ad_tiles = []
            for k, (dy, dx) in enumerate(REPS):
                lo_r, n_r, lo_c, n_c = _grid_geom(dy, dx)
                ad = adpool.tile([128, n_r, n_c], bf16, tag=f"ad{k}")
                ctr = xe_t[:, lo_r + 5 : lo_r + 5 + n_r, lo_c + 6 : lo_c + 6 + n_c]
                if dx % 2 == 0:
                    sft = xe_t[
                        :,
                        lo_r + dy + 5 : lo_r + dy + 5 + n_r,
                        lo_c + dx + 6 : lo_c + dx + 6 + n_c,
                    ]
                else:
                    # xo col j holds x col j-5  (x shifted one col left)
                    sft = xo_t[
                        :,
                        lo_r + dy + 5 : lo_r + dy + 5 + n_r,
                        lo_c + dx + 5 : lo_c + dx + 5 + n_c,
                    ]
                nc.vector._custom_dve(absdiff, out=ad[:], in0=ctr, in1=sft)
                ad_tiles.append((ad, lo_r, lo_c))

            for nb in range(NB):
                r0 = nb * 16
                dA = psum1.tile([96, NBW], f32, tag="dA")
                dB = psum1.tile([96, NBW], f32, tag="dB")
                # + windows -> dB rows (g, k): offsets 13+k
                for k, (dy, dx) in enumerate(REPS):
                    ad, lo_r, lo_c = ad_tiles[k]
                    wp = ad[:, -lo_r + r0 : -lo_r + r0 + 16, -lo_c : -lo_c + GW]
                    nc.tensor.matmul(
                        dB[:],
                        ls_t[:, (2 * k) * 96 : (2 * k + 1) * 96],
                        wp,
                        start=(k == 0),
                        stop=(k == 11),
                    )
                # - windows -> dA rows (g, 11-k): offsets 11-k
                for k, (dy, dx) in enumerate(REPS):
                    ad, lo_r, lo_c = ad_tiles[k]
                    wm = ad[
                        :,
                        -lo_r - dy + r0 : -lo_r - dy + r0 + 16,
                        -lo_c - dx : -lo_c - dx + GW,
                    ]
                    nc.tensor.matmul(
                        dA[:],
                        ls_t[:, (2 * k + 1) * 96 : (2 * k + 2) * 96],
                        wm,
                        start=(k == 0),
                        stop=(k == 11),
                    )
                dAs = dwork.tile([96, NBW], f32, tag="dAs")
                nc.scalar.copy(dAs[:], dA[:])
                dBs = dwork.tile([96, NBW], f32, tag="dBs")
                nc.scalar.copy(dBs[:], dB[:])

                h = psum2.tile([80, NBW], f32, tag="h")
                nc.tensor.matmul(h[:], l1_t[:, 0:80], dAs[:], start=True, stop=False)
                nc.tensor.matmul(h[:], l1_t[:, 80:160], dBs[:], start=False, stop=True)
                hs = dwork.tile([80, NBW], f32, tag="hs")
                nc.scalar.activation(hs[:], h[:], AF.Gelu, bias=bias_t[0:80, 0:1])

                zA = psum2.tile([104, NBW], f32, tag="zA")
                nc.tensor.matmul(zA[:], l2_t[:, 0:104], hs[:], start=True, stop=True)
                zB = psum2.tile([96, NBW], f32, tag="zB")
                nc.tensor.matmul(zB[:], l2_t[:, 104:200], hs[:], start=True, stop=True)

                oA = opool.tile([104, NBW], f32, tag="oA")
                nc.scalar.activation(oA[:], zA[:], AF.Relu, bias=bias_t[0:104, 1:2])
                oB = opool.tile([96, NBW], f32, tag="oB")
                nc.scalar.activation(oB[:], zB[:], AF.Relu, bias=bias_t[0:96, 2:3])

                for g in range(G):
                    nc.sync.dma_start(
                        out_d[g, 0:13, r0 : r0 + 16, :],
                        oA[g * 13 : (g + 1) * 13, :].rearrange(
                            "p (a b) -> p a b", b=GW
                        ),
                    )
                    nc.sync.dma_start(
                        out_d[g, 13:25, r0 : r0 + 16, :],
                        oB[g * 12 : (g + 1) * 12, :].rearrange(
                            "p (a b) -> p a b", b=GW
                        ),
                    )
    nc.compile()
    return nc


def _get_module(iters=1):
    with _lock:
        if iters not in _cached:
            _cached[iters] = _build_module(iters)
        return _cached[iters]


def _consts():
    return {}


def _pack_weights(W1, b1, W2, b2):
    ls = np.zeros((128, 24 * 96), np.float32)
    for k in range(12):
        lp = ls[:, (2 * k) * 96 : (2 * k + 1) * 96]
        lm = ls[:, (2 * k + 1) * 96 : (2 * k + 2) * 96]
        for g in range(G):
            for c in range(C):
                p = g * 16 + c
                lp[p, g * 12 + k] = 1.0
                lm[p, g * 12 + (11 - k)] = 1.0
    l1 = np.zeros((96, 160), np.float32)
    l2 = np.zeros((80, 200), np.float32)
    for g in range(G):
        l1[g * 12 : (g + 1) * 12, g * 10 : (g + 1) * 10] = W1[0:12, :]
        l1[g * 12 : (g + 1) * 12, 80 + g * 10 : 80 + (g + 1) * 10] = W1[13:25, :]
        l2[g * 10 : (g + 1) * 10, g * 13 : (g + 1) * 13] = W2[:, 0:13]
        l2[g * 10 : (g + 1) * 10, 104 + g * 12 : 104 + (g + 1) * 12] = W2[:, 13:25]
    bias = np.zeros((104, 3), np.float32)
    bias[0:80, 0] = np.tile(b1, G)
    bias[0:104, 1] = np.tile(b2[0:13], G)
    bias[0:96, 2] = np.tile(b2[13:25], G)
    return ls.astype(BF16), l1, l2, bias


def _build_in_maps(x, W1, b1, W2, b2):
    ls, l1, l2, bias = _pack_weights(W1, b1, W2, b2)
    # reflect-pad full image: rows +-5, cols +-7 (extra col for the odd tile)
    xp = np.pad(x, ((0, 0), (0, 0), (5, 5), (7, 7)), mode="reflect")
    in_maps = []
    for core in range(N_CORES):
        b, half = divmod(core, 2)
        r0 = 128 * half
        xe = np.empty((128, XR, XC), np.float32)
        xo = np.empty((128, XR, XC), np.float32)
        for g in range(G):
            # x tile col j holds image col XLO_C+j = 32g-6+j -> xp col 32g+1+j
            xe[g * 16 : (g + 1) * 16] = xp[b, :, r0 : r0 + XR, 32 * g + 1 : 32 * g + 1 + XC]
            xo[g * 16 : (g + 1) * 16] = xp[b, :, r0 : r0 + XR, 32 * g + 2 : 32 * g + 2 + XC]
        in_maps.append(
            {
                "xe": xe.astype(BF16),
                "xo": xo.astype(BF16),
                "ls": ls,
                "l1": l1,
                "l2": l2,
                "bias": bias,
            }
        )
    return in_maps


def _loss_from_dl1(dl1, y):
    yp = np.pad(y[:, 0], ((0, 0), (5, 5), (5, 5)), mode="reflect")
    n_pos = 0
    a_sum = 0.0  # sum over matches of relu(d - alpha)
    b_sum = 0.0  # sum over mismatches of relu(beta - d)
    for i, (dy, dx) in enumerate(OFFS):
        eq = yp[:, 5 + dy : 5 + dy + H, 5 + dx : 5 + dx + W] == y[:, 0]
        d = dl1[..., i]
        n_pos += int(eq.sum())
        a_sum += float(np.maximum(d - ALPHA, 0.0).sum(dtype=np.float64, where=eq))
        b_sum += float(np.maximum(BETA - d, 0.0).sum(dtype=np.float64, where=~eq))
    numel = B * H * W * 25
    n_neg = numel - n_pos
    loss = (n_pos * b_sum + n_neg * a_sum) / float(numel) ** 2
    return np.float32(loss)


def _run(in_maps, iters=1):
    from concourse import bass_utils

    nc = _get_module(iters)
    res = bass_utils.run_bass_kernel_spmd(
        nc, in_maps, core_ids=list(range(N_CORES))
    )
    return res.results


def kernel(x, y, W1, b1, W2, b2):
    x = np.asarray(x, np.float32)
    y = np.asarray(y, np.int32)
    in_maps = _build_in_maps(
        x,
        np.asarray(W1, np.float32),
        np.asarray(b1, np.float32),
        np.asarray(W2, np.float32),
        np.asarray(b2, np.float32),
    )
    results = _run(in_maps)
    dl1 = np.empty((B, H, W, 25), np.float32)
    for core in range(N_CORES):
        out = results[core]["out"]  # [G, 25, ROWS, GW]
        b, half = divmod(core, 2)
        r0 = 128 * half
        # [g, o, r, c] -> [r, g*32+c, o]
        dl1[b, r0 : r0 + ROWS] = out.transpose(2, 0, 3, 1).reshape(ROWS, W, 25)
    loss = _loss_from_dl1(dl1, y)
    return dl1, loss


# revision 8
# speedup vs baseline: 36.0402x; 36.0402x over previous
# Trainium2 Bass kernel for nn_DistLoss: dist = 25-neighbor channel-L1
# distances -> tiny MLP (25->10->25, exact gelu) -> relu = dist_l1;
# loss assembled from dist_l1 + label-agreement mask.
#
# Sharding: data-parallel over (batch, image half): core k handles batch k//2,
# rows 128*(k%2) .. +128.  Inside a core, SBUF partitions = (column-group g,
# channel c): p = g*16 + c, so the channel reduction is a PE matmul over the
# partition axis and all neighbor shifts are pure free-dim AP offsets into a
# reflection-padded x tile prepared host-side.  |x - shift(x)| is computed for
# only 12 of the 24 non-center offsets; the mirror offset reuses the same
# absdiff grid at a shifted window (|a-b| symmetry).  The MLP runs on PE with
# block-diagonal packed weights (8 column-groups at once); gelu/relu/bias on
# the scalar engine.  dist_l1 is written HBM-contiguous in a device-friendly
# [g, o, r, c] layout and transposed to [B,H,W,25] on host.  The scalar loss
# factorizes as (n_pos*S_mismatch + n_neg*S_match)/numel^2 with plain sums, so
# it is assembled on host from dist_l1 and y.
import threading

import numpy as np
import ml_dtypes

B, C, H, W = 4, 16, 256, 256
ALPHA, BETA = 0.5, 2.0
N_CORES = 8
ROWS = 128           # output rows per core
G, GW = 8, 32        # column groups x width
XLO_R, XHI_R = -5, 132
XLO_C, XHI_C = -6, 37
XR = XHI_R - XLO_R + 1   # 138
XC = XHI_C - XLO_C + 1   # 44
NB = 8               # n-superblocks (each 16 rows x 32 cols = 512 pixels)
NBW = 512
BF16 = ml_dtypes.bfloat16


def _sorted_offsets():
    K = 11
    c0 = K // 2
    offs = set()
    for r in (1, 2, 5):
        for dy in (-r, 0, r):
            for dx in (-r, 0, r):
                offs.add((c0 + dy) * K + (c0 + dx))
    return [(v // K - c0, v % K - c0) for v in sorted(offs)]


OFFS = _sorted_offsets()          # center at index 12; OFFS[24-i] == -OFFS[i]
REPS = OFFS[13:]                  # 12 representatives (positive half)


def _grid_geom(dy, dx):
    """absdiff grid for rep (dy,dx): covers +window [0..127]x[0..31] and
    -window (grid at p-delta); col start/width padded even for DVE bf16
    2x-mode alignment."""
    lo_r = -dy
    n_r = 128 + dy
    if dx > 0:
        lo_c, hi_c = -dx, 31
    else:
        lo_c, hi_c = 0, 31 - dx
    if lo_c % 2 != 0:
        lo_c -= 1
    if (hi_c - lo_c + 1) % 2 != 0:
        hi_c += 1
    return lo_r, n_r, lo_c, hi_c - lo_c + 1


_lock = threading.Lock()
_cached = {}
_absdiff_op = None


def _get_absdiff_op():
    """Register a fused |a-b| custom DVE op: one Vector-engine pass instead of
    subtract + abs (abs_max is not a valid TRN2 TensorTensor ALU op)."""
    global _absdiff_op
    if _absdiff_op is not None:
        return _absdiff_op
    from concourse import dve_ops as dvo
    from concourse.dve_spec import Spec, Src0, Src1, maxx, lower, _has_src1
    from concourse.dve_uop import DveOpSpec

    name = "ABS_DIFF_ANT"
    spec = Spec(
        body=maxx(Src0 - Src1, Src1 - Src0),
        reference=lambda in0, in1, s0, s1, imm2: np.abs(
            in0.astype(np.float32) - in1.astype(np.float32)
        ),
    )
    op = dvo.DveOp(name, spec, subdim=False, uops_sha={})
    if name not in dvo._SUB_OPCODE_FOR_NAME:
        row = max(dvo._SUB_OPCODE_FOR_NAME.values()) + 1
        assert row < 0x20
        dvo._SUB_OPCODE_FOR_NAME[name] = row
        dvo.OPS.append(op)
        dvo.CUSTOM_DVE_SPECS[name] = spec
    row = dvo._SUB_OPCODE_FOR_NAME[name]
    for ver in ("v3",):
        s = DveOpSpec(
            name=name, opcode=row, uops=lower(spec, ver=ver), rd1_en=_has_src1(spec)
        ).sha(ver)
        op.uops_sha[ver] = s
    _absdiff_op = op
    return op


def _build_module(iters=1):
    import concourse.bacc as bacc
    import concourse.mybir as mybir
    from concourse.tile import TileContext
    from contextlib import ExitStack

    f32 = mybir.dt.float32
    bf16 = mybir.dt.bfloat16
    AF = mybir.ActivationFunctionType
    ALU = mybir.AluOpType

    absdiff = _get_absdiff_op()
    nc = bacc.Bacc(
        "TRN2",
        target_bir_lowering=False,
        debug=False,
        enable_asserts=False,
        num_devices=N_CORES,
    )

    xe_d = nc.dram_tensor("xe", [128, XR, XC], bf16, kind="ExternalInput").ap()
    xo_d = nc.dram_tensor("xo", [128, XR, XC], bf16, kind="ExternalInput").ap()
    ls_d = nc.dram_tensor("ls", [128, 24 * 96], bf16, kind="ExternalInput").ap()
    l1_d = nc.dram_tensor("l1", [96, 160], f32, kind="ExternalInput").ap()
    l2_d = nc.dram_tensor("l2", [80, 200], f32, kind="ExternalInput").ap()
    bias_d = nc.dram_tensor("bias", [104, 3], f32, kind="ExternalInput").ap()
    out_d = nc.dram_tensor("out", [G, 25, ROWS, GW], f32, kind="ExternalOutput").ap()

    with TileContext(nc) as tc, ExitStack() as ctx:
        const = ctx.enter_context(tc.tile_pool(name="const", bufs=1))
        xpool = ctx.enter_context(tc.tile_pool(name="x", bufs=1))
        adpool = ctx.enter_context(tc.tile_pool(name="ad", bufs=1))
        dwork = ctx.enter_context(tc.tile_pool(name="dwork", bufs=2))
        opool = ctx.enter_context(tc.tile_pool(name="o", bufs=3))
        psum1 = ctx.enter_context(tc.tile_pool(name="ps1", bufs=1, space="PSUM"))
        psum2 = ctx.enter_context(tc.tile_pool(name="ps2", bufs=2, space="PSUM"))

        ls_t = const.tile([128, 24 * 96], bf16)
        nc.sync.dma_start(ls_t[:], ls_d[:])
        l1_t = const.tile([96, 160], f32)
        nc.sync.dma_start(l1_t[:], l1_d[:])
        l2_t = const.tile([80, 200], f32)
        nc.sync.dma_start(l2_t[:], l2_d[:])
        bias_t = const.tile([104, 3], f32)
        nc.sync.dma_start(bias_t[:], bias_d[:])

        for _ in range(iters):
            xe_t = xpool.tile([128, XR, XC], bf16, tag="xe")
            nc.sync.dma_start(xe_t[:], xe_d[:])
            xo_t = xpool.tile([128, XR, XC], bf16, tag="xo")
            nc.sync.dma_start(xo_t[:], xo_d[:])

            ad_tiles = []
            for k, (dy, dx) in enumerate(REPS):
                lo_r, n_r, lo_c, n_c = _grid_geom(dy, dx)
                ad = adpool.tile([128, n_r, n_c], bf16, tag=f"ad{k}")
                ctr = xe_t[:, lo_r + 5 : lo_r + 5 + n_r, lo_c + 6 : lo_c + 6 + n_c]
                if dx % 2 == 0:
                    sft = xe_t[
                        :,
                        lo_r + dy + 5 : lo_r + dy + 5 + n_r,
                        lo_c + dx + 6 : lo_c + dx + 6 + n_c,
                    ]
                else:
                    # xo col j holds x col j-5  (x shifted one col left)
                    sft = xo_t[
                        :,
                        lo_r + dy + 5 : lo_r + dy + 5 + n_r,
                        lo_c + dx + 5 : lo_c + dx + 5 + n_c,
                    ]
                nc.vector._custom_dve(absdiff, out=ad[:], in0=ctr, in1=sft)
                ad_tiles.append((ad, lo_r, lo_c))

            for nb in range(NB):
                r0 = nb * 16
                dA = psum1.tile([96, NBW], f32, tag="dA")
                dB = psum1.tile([96, NBW], f32, tag="dB")
                # + windows -> dB rows (g, k): offsets 13+k
                for k, (dy, dx) in enumerate(REPS):
                    ad, lo_r, lo_c = ad_tiles[k]
                    wp = ad[:, -lo_r + r0 : -lo_r + r0 + 16, -lo_c : -lo_c + GW]
                    nc.tensor.matmul(
                        dB[:],
                        ls_t[:, (2 * k) * 96 : (2 * k + 1) * 96],
                        wp,
                        start=(k == 0),
                        stop=(k == 11),
                    )
                # - windows -> dA rows (g, 11-k): offsets 11-k
                for k, (dy, dx) in enumerate(REPS):
                    ad, lo_r, lo_c = ad_tiles[k]
                    wm = ad[
                        :,
                        -lo_r - dy + r0 : -lo_r - dy + r0 + 16,
                        -lo_c - dx : -lo_c - dx + GW,
                    ]
                    nc.tensor.matmul(
                        dA[:],
                        ls_t[:, (2 * k + 1) * 96 : (2 * k + 2) * 96],
                        wm,
                        start=(k == 0),
                        stop=(k == 11),
                    )
                dAs = dwork.tile([96, NBW], f32, tag="dAs")
                nc.scalar.copy(dAs[:], dA[:])
                dBs = dwork.tile([96, NBW], f32, tag="dBs")
                nc.scalar.copy(dBs[:], dB[:])

                h = psum2.tile([80, NBW], f32, tag="h")
                nc.tensor.matmul(h[:], l1_t[:, 0:80], dAs[:], start=True, stop=False)
                nc.tensor.matmul(h[:], l1_t[:, 80:160], dBs[:], start=False, stop=True)
                hs = dwork.tile([80, NBW], f32, tag="hs")
                nc.scalar.activation(hs[:], h[:], AF.Gelu, bias=bias_t[0:80, 0:1])

                zA = psum2.tile([104, NBW], f32, tag="zA")
                nc.tensor.matmul(zA[:], l2_t[:, 0:104], hs[:], start=True, stop=True)
                zB = psum2.tile([96, NBW], f32, tag="zB")
                nc.tensor.matmul(zB[:], l2_t[:, 104:200], hs[:], start=True, stop=True)

                oA = opool.tile([104, NBW], f32, tag="oA")
                nc.scalar.activation(oA[:], zA[:], AF.Relu, bias=bias_t[0:104, 1:2])
                oB = opool.tile([96, NBW], f32, tag="oB")
                nc.scalar.activation(oB[:], zB[:], AF.Relu, bias=bias_t[0:96, 2:3])

                for g in range(G):
                    nc.sync.dma_start(
                        out_d[g, 0:13, r0 : r0 + 16, :],
                        oA[g * 13 : (g + 1) * 13, :].rearrange(
                            "p (a b) -> p a b", b=GW
                        ),
                    )
                    nc.sync.dma_start(
                        out_d[g, 13:25, r0 : r0 + 16, :],
                        oB[g * 12 : (g + 1) * 12, :].rearrange(
                            "p (a b) -> p a b", b=GW
                        ),
                    )
    nc.compile()
    return nc


def _get_module(iters=1):
    with _lock:
        if iters not in _cached:
            _cached[iters] = _build_module(iters)
        return _cached[iters]


def _consts():
    return {}


def _pack_weights(W1, b1, W2, b2):
    ls = np.zeros((128, 24 * 96), np.float32)
    for k in range(12):
        lp = ls[:, (2 * k) * 96 : (2 * k + 1) * 96]
        lm = ls[:, (2 * k + 1) * 96 : (2 * k + 2) * 96]
        for g in range(G):
            for c in range(C):
                p = g * 16 + c
                lp[p, g * 12 + k] = 1.0
                lm[p, g * 12 + (11 - k)] = 1.0
    l1 = np.zeros((96, 160), np.float32)
    l2 = np.zeros((80, 200), np.float32)
    for g in range(G):
        l1[g * 12 : (g + 1) * 12, g * 10 : (g + 1) * 10] = W1[0:12, :]
        l1[g * 12 : (g + 1) * 12, 80 + g * 10 : 80 + (g + 1) * 10] = W1[13:25, :]
        l2[g * 10 : (g + 1) * 10, g * 13 : (g + 1) * 13] = W2[:, 0:13]
        l2[g * 10 : (g + 1) * 10, 104 + g * 12 : 104 + (g + 1) * 12] = W2[:, 13:25]
    bias = np.zeros((104, 3), np.float32)
    bias[0:80, 0] = np.tile(b1, G)
    bias[0:104, 1] = np.tile(b2[0:13], G)
    bias[0:96, 2] = np.tile(b2[13:25], G)
    return ls.astype(BF16), l1, l2, bias


def _build_in_maps(x, W1, b1, W2, b2):
    ls, l1, l2, bias = _pack_weights(W1, b1, W2, b2)
    # reflect-pad full image: rows +-5, cols +-7 (extra col for the odd tile)
    xp = np.pad(x, ((0, 0), (0, 0), (5, 5), (7, 7)), mode="reflect")
    in_maps = []
    for core in range(N_CORES):
        b, half = divmod(core, 2)
        r0 = 128 * half
        xe = np.empty((128, XR, XC), np.float32)
        xo = np.empty((128, XR, XC), np.float32)
        for g in range(G):
            # x tile col j holds image col XLO_C+j = 32g-6+j -> xp col 32g+1+j
            xe[g * 16 : (g + 1) * 16] = xp[b, :, r0 : r0 + XR, 32 * g + 1 : 32 * g + 1 + XC]
            xo[g * 16 : (g + 1) * 16] = xp[b, :, r0 : r0 + XR, 32 * g + 2 : 32 * g + 2 + XC]
        in_maps.append(
            {
                "xe": xe.astype(BF16),
                "xo": xo.astype(BF16),
                "ls": ls,
                "l1": l1,
                "l2": l2,
                "bias": bias,
            }
        )
    return in_maps


def _loss_from_dl1(dl1, y):
    yp = np.pad(y[:, 0], ((0, 0), (5, 5), (5, 5)), mode="reflect")
    n_pos = 0
    a_sum = 0.0  # sum over matches of relu(d - alpha)
    b_sum = 0.0  # sum over mismatches of relu(beta - d)
    for i, (dy, dx) in enumerate(OFFS):
        eq = yp[:, 5 + dy : 5 + dy + H, 5 + dx : 5 + dx + W] == y[:, 0]
        d = dl1[..., i]
        n_pos += int(eq.sum())
        a_sum += float(np.maximum(d - ALPHA, 0.0).sum(dtype=np.float64, where=eq))
        b_sum += float(np.maximum(BETA - d, 0.0).sum(dtype=np.float64, where=~eq))
    numel = B * H * W * 25
    n_neg = numel - n_pos
    loss = (n_pos * b_sum + n_neg * a_sum) / float(numel) ** 2
    return np.float32(loss)


_runner_cache = {}


def _get_runner(iters=1):
    """Build the jitted 8-core executor ONCE per NEFF variant.  (bass_utils'
    run_bass_kernel_spmd re-creates jax.jit per call, which re-traces and
    re-loads the NEFF every time.)"""
    if iters in _runner_cache:
        return _runner_cache[iters]
    import jax
    import concourse.mybir as mybir
    from concourse import bass2jax as b2j
    from jax.experimental.shard_map import shard_map
    from jax.sharding import Mesh, PartitionSpec

    b2j.install_neuronx_cc_hook()
    nc = _get_module(iters)

    partition_name = nc.partition_id_tensor.name if nc.partition_id_tensor else None
    in_names, out_names, out_avals, zero_outs = [], [], [], []
    for alloc in nc.m.functions[0].allocations:
        if not isinstance(alloc, mybir.MemoryLocationSet):
            continue
        name = alloc.memorylocations[0].name
        if alloc.kind == "ExternalInput":
            if name != partition_name:
                in_names.append(name)
        elif alloc.kind == "ExternalOutput":
            out_names.append(name)
            shape = tuple(alloc.tensor_shape)
            dtype = mybir.dt.np(alloc.dtype)
            out_avals.append(jax.core.ShapedArray(shape, dtype))
            zero_outs.append(np.zeros(shape, dtype))
    n_params = len(in_names)
    all_names = in_names + out_names
    if partition_name is not None:
        all_names = all_names + [partition_name]

    def _body(*args):
        operands = list(args)
        if partition_name is not None:
            operands.append(b2j.partition_id_tensor())
        outs = b2j._bass_exec_p.bind(
            *operands,
            out_avals=tuple(out_avals),
            in_names=tuple(all_names),
            out_names=tuple(out_names),
            lowering_input_output_aliases=(),
            sim_require_finite=True,
            sim_require_nnan=True,
            nc=nc,
        )
        return tuple(outs)

    devices = jax.devices()[:N_CORES]
    mesh = Mesh(np.asarray(devices), ("core",))
    n_outs = len(out_names)
    sharded = jax.jit(
        shard_map(
            _body,
            mesh=mesh,
            in_specs=(PartitionSpec("core"),) * (n_params + n_outs),
            out_specs=(PartitionSpec("core"),) * n_outs,
            check_rep=False,
        ),
        donate_argnums=tuple(range(n_params, n_params + n_outs)),
        keep_unused=True,
    )

    def run(in_maps):
        per_core = [[np.asarray(m[n]) for n in in_names] for m in in_maps]
        concat_in = [
            np.concatenate([per_core[c][i] for c in range(N_CORES)], axis=0)
            for i in range(n_params)
        ]
        concat_zeros = [
            np.zeros((N_CORES * z.shape[0], *z.shape[1:]), z.dtype)
            for z in zero_outs
        ]
        out_arrs = sharded(*concat_in, *concat_zeros)
        out_arrs = [np.asarray(a) for a in out_arrs]
        return [
            {
                name: out_arrs[i].reshape(N_CORES, *out_avals[i].shape)[c]
                for i, name in enumerate(out_names)
            }
            for c in range(N_CORES)
        ]

    _runner_cache[iters] = run
    return run


def _run(in_maps, iters=1):
    return _get_runner(iters)(in_maps)


def kernel(x, y, W1, b1, W2, b2):
    x = np.asarray(x, np.float32)
    y = np.asarray(y, np.int32)
    in_maps = _build_in_maps(
        x,
        np.asarray(W1, np.float32),
        np.asarray(b1, np.float32),
        np.asarray(W2, np.float32),
        np.asarray(b2, np.float32),
    )
    results = _run(in_maps)
    dl1 = np.empty((B, H, W, 25), np.float32)
    for core in range(N_CORES):
        out = results[core]["out"]  # [G, 25, ROWS, GW]
        b, half = divmod(core, 2)
        r0 = 128 * half
        # [g, o, r, c] -> [r, g*32+c, o]
        dl1[b, r0 : r0 + ROWS] = out.transpose(2, 0, 3, 1).reshape(ROWS, W, 25)
    loss = _loss_from_dl1(dl1, y)
    return dl1, loss
